# revision 11
# baseline (speedup 1.0000x reference)
"""Trainium2 Bass kernel for nn_CGCA_branch (gnn_message_passing).

Math: the reference applies 1x1 convs (C->CA, grouped CA->CA), global average
pool, fc1, adjacency-softmax matmul, relu, fc2, sigmoid.  Every op between x
and the relu is linear, and the global average pool commutes with the 1x1
convs, so the whole prefix collapses to

    f1[n, :] = Wcomb @ sum_s(x[n, :, s]),   Wcomb = fc1_w @ M2 @ (w1 / S)

with M2 the block-diagonal form of the grouped conv.  The kernel therefore
only needs a 411 MB spatial-sum reduction of x (HBM-bound) plus tiny matmuls.

Sharding: pure data parallel - batch 64 split into 8 shards of 8 samples,
one per NeuronCore; weights replicated.
"""

import numpy as np

import concourse.bass as bass
import concourse.bacc as bacc
import concourse.tile as tile
from concourse import mybir
from concourse.bass_utils import run_bass_kernel_spmd
from concourse.tile import TileContext
from contextlib import ExitStack

# ---- problem constants (hardcoded per harness contract) ----
N, C, H, W = 64, 512, 56, 56
S = H * W                      # 3136 spatial positions
J, CA, G = 17, 272, 16
NCORES = 8
NL = N // NCORES               # 8 samples per core
CT = C // 128                  # 4 channel chunks of 128
NEG = -9e15

_ADJ = np.array([
    [1,1,0,0,0,0,0,0,0,0,0,0,0,0,0,0,0],[1,1,1,0,0,0,0,0,0,0,0,0,0,0,0,0,0],
    [0,1,1,0,0,0,1,0,0,0,0,0,0,0,0,0,0],[0,0,0,1,1,0,1,0,0,0,0,0,0,0,0,0,0],
    [0,0,0,1,1,1,0,0,0,0,0,0,0,0,0,0,0],[0,0,0,0,1,1,0,0,0,0,0,0,0,0,0,0,0],
    [0,0,1,1,0,0,1,1,0,0,0,0,0,0,0,0,0],[0,0,0,0,0,0,1,1,1,0,0,0,0,0,0,0,0],
    [0,0,0,0,0,0,0,1,1,0,0,1,1,0,0,0,1],[0,0,0,0,0,0,0,0,0,1,0,0,0,0,0,0,1],
    [0,0,0,0,0,0,0,0,0,0,1,1,0,0,0,0,0],[0,0,0,0,0,0,0,0,0,0,1,1,1,0,0,0,0],
    [0,0,0,0,0,0,0,0,1,0,0,1,1,0,0,0,0],[0,0,0,0,0,0,0,0,1,0,0,0,0,1,1,0,0],
    [0,0,0,0,0,0,0,0,0,0,0,0,0,1,1,1,0],[0,0,0,0,0,0,0,0,0,0,0,0,0,0,1,1,0],
    [0,0,0,0,0,0,0,0,1,1,0,0,0,0,0,0,1]], dtype=np.int32)
NZ_IDX = np.flatnonzero(_ADJ)  # 49 entries

F32 = mybir.dt.float32
_NC_CACHE = {}


def _build_nc() -> bass.Bass:
    nc = bacc.Bacc(None, enable_partition_id=False)
    x_d = nc.declare_dram_parameter("x", [NL, C, S], F32, isOutput=False)
    wct_d = nc.declare_dram_parameter("wct", [128, CT, J], F32, isOutput=False)
    emat_d = nc.declare_dram_parameter("emat", [J, J], F32, isOutput=False)
    ematt_d = nc.declare_dram_parameter("ematt", [J, J], F32, isOutput=False)
    fc2t_d = nc.declare_dram_parameter("fc2t", [J, C], F32, isOutput=False)
    out_d = nc.declare_dram_parameter("out", [NL, C], F32, isOutput=True)

    with TileContext(nc) as tc, ExitStack() as ctx:
        xpool = ctx.enter_context(tc.tile_pool(name="xpool", bufs=6))
        singles = ctx.enter_context(tc.tile_pool(name="singles", bufs=1))
        smalls = ctx.enter_context(tc.tile_pool(name="smalls", bufs=3))
        resp = ctx.enter_context(tc.tile_pool(name="resp", bufs=NL))
        psum = ctx.enter_context(tc.tile_pool(name="psum", bufs=2, space="PSUM"))

        # ---- replicated weights / adjacency prep (tiny, one-time).
        # SWDGE queue so the SP HWDGE ring carries only the x stream.
        wct_sb = singles.tile([128, CT, J], F32)
        nc.gpsimd.dma_start(out=wct_sb, in_=wct_d[:, :, :])
        fc2t_sb = singles.tile([J, C], F32)
        nc.gpsimd.dma_start(out=fc2t_sb, in_=fc2t_d[:, :])
        e_sb = singles.tile([J, J], F32)
        nc.gpsimd.dma_start(out=e_sb, in_=emat_d[:, :])
        et_sb = singles.tile([J, J], F32)
        nc.gpsimd.dma_start(out=et_sb, in_=ematt_d[:, :])

        # softmax over rows of E: adj[i,j] = exp(E[i,j]) / rs[i].
        # We keep exp(E^T) as the matmul lhsT and fold 1/rs in afterwards.
        a_sb = singles.tile([J, J], F32)
        nc.scalar.activation(out=a_sb, in_=e_sb,
                             func=mybir.ActivationFunctionType.Exp)
        at_sb = singles.tile([J, J], F32)
        nc.scalar.activation(out=at_sb, in_=et_sb,
                             func=mybir.ActivationFunctionType.Exp)
        rs_sb = singles.tile([J, 1], F32)
        nc.vector.reduce_sum(out=rs_sb, in_=a_sb, axis=mybir.AxisListType.X)
        rrs_sb = singles.tile([J, 1], F32)
        nc.vector.reciprocal(out=rrs_sb, in_=rs_sb)

        # ---- stream x, spatial-sum per (sample, channel-chunk) ----
        # The tail-critical final chunks are split into smaller pieces so the
        # last reduce after the last DMA is ~1us instead of ~3us; the partial
        # sums are folded into extra PSUM-accumulated matmuls below.
        xm_sb = singles.tile([128, CT, NL], F32)        # xm[p, ct, n]
        stage = singles.tile([128, 8], F32)             # split-piece partials
        scratch = singles.tile([128, S], F32)           # dummy out for ACT accum
        xv = x_d[:, :, :].rearrange("n (ct p) s -> n p ct s", p=128)

        n_pieces = {(NL - 1, CT - 2): 2, (NL - 1, CT - 1): 4}
        f1_ops = {n: [] for n in range(NL)}   # (lhsT, rhs) accumulation list
        stage_col = 0

        def emit_reduce(n, ct, use_dve):
            nonlocal stage_col
            pieces = n_pieces.get((n, ct), 1)
            w = S // pieces
            for pi in range(pieces):
                xt = xpool.tile([128, w], F32, tag="xt")
                # alternate the two DMA rings (SP HWDGE / Pool SWDGE) so
                # descriptor generation is never the stream bottleneck
                dma_eng = nc.sync if (n * CT + ct) % 2 == 0 else nc.gpsimd
                dma_eng.dma_start(out=xt, in_=xv[n, :, ct, pi * w:(pi + 1) * w])
                if pieces == 1:
                    dst = xm_sb[:, ct, n:n + 1]
                else:
                    dst = stage[:, stage_col:stage_col + 1]
                    stage_col += 1
                f1_ops[n].append((wct_sb[:, ct, :], dst))
                if use_dve:
                    nc.vector.reduce_sum(out=dst, in_=xt,
                                         axis=mybir.AxisListType.X)
                else:
                    nc.scalar.activation(
                        out=scratch[:, :w], in_=xt,
                        func=mybir.ActivationFunctionType.Copy,
                        accum_out=dst)

        def emit_sample_chain(n):
            # f1[:, n] = Wcomb @ xm[:, n]  (accumulate over chunks/pieces)
            ops = f1_ops[n]
            f1_ps = psum.tile([J, 1], F32, tag="f1")
            for i, (lhsT, rhs) in enumerate(ops):
                nc.tensor.matmul(f1_ps, lhsT=lhsT, rhs=rhs,
                                 start=(i == 0), stop=(i == len(ops) - 1))
            f1_sb = smalls.tile([J, 1], F32, tag="f1s")
            nc.scalar.copy(out=f1_sb, in_=f1_ps)
            # gc = relu(adj @ f1) via exp(E^T) lhsT and 1/rs scaling
            gc_ps = psum.tile([J, 1], F32, tag="gc")
            nc.tensor.matmul(gc_ps, lhsT=at_sb, rhs=f1_sb, start=True,
                             stop=True)
            gc_sb = smalls.tile([J, 1], F32, tag="gcs")
            nc.vector.tensor_scalar(out=gc_sb, in0=gc_ps, scalar1=rrs_sb,
                                    scalar2=0.0, op0=mybir.AluOpType.mult,
                                    op1=mybir.AluOpType.max)
            # out = sigmoid(gc.T @ fc2t), in two column halves so the final
            # matmul / sigmoid / store pipeline on the kernel tail
            res_sb = resp.tile([1, C], F32, tag="res")
            half = C // 2
            for h in range(2):
                o_ps = psum.tile([1, half], F32, tag="o")
                nc.tensor.matmul(o_ps, lhsT=gc_sb,
                                 rhs=fc2t_sb[:, h * half:(h + 1) * half],
                                 start=True, stop=True)
                nc.scalar.activation(out=res_sb[:, h * half:(h + 1) * half],
                                     in_=o_ps,
                                     func=mybir.ActivationFunctionType.Sigmoid)
            return res_sb

        results = []
        for n in range(NL):
            for ct in range(CT):
                emit_reduce(n, ct, use_dve=(ct % 2 == 0))
            results.append(emit_sample_chain(n))
        # output DMAs last so they never stall the x stream on the SP ring
        for n, res_sb in enumerate(results):
            nc.sync.dma_start(out=out_d[n:n + 1, :], in_=res_sb)

    return nc


def _get_nc() -> bass.Bass:
    if "nc" not in _NC_CACHE:
        nc = _build_nc()
        nc.finalize()
        _NC_CACHE["nc"] = nc
    return _NC_CACHE["nc"]


def _prep_inputs(x, e, w1, w2, fc1_w, fc2_w):
    """Host-side shard + weight fold (layout prep only; heavy math on device)."""
    x = np.ascontiguousarray(np.asarray(x, dtype=np.float32)).reshape(N, C, S)

    # fold conv1 / grouped-conv2 / fc1 / (1/S mean) into one [J, C] matrix
    w1d = np.asarray(w1, dtype=np.float64)
    w2g = np.asarray(w2, dtype=np.float64).reshape(G, J, J)
    m2 = np.zeros((CA, CA), dtype=np.float64)
    for g in range(G):
        m2[g * J:(g + 1) * J, g * J:(g + 1) * J] = w2g[g]
    wcomb = np.asarray(fc1_w, np.float64) @ m2 @ (w1d / S)      # [J, C]
    wct = np.ascontiguousarray(
        wcomb.T.reshape(CT, 128, J).transpose(1, 0, 2)).astype(np.float32)

    emat = np.full((J * J,), NEG, dtype=np.float32)
    emat[NZ_IDX] = np.asarray(e, dtype=np.float32)[0]
    emat = emat.reshape(J, J)
    ematt = np.ascontiguousarray(emat.T)
    fc2t = np.ascontiguousarray(np.asarray(fc2_w, dtype=np.float32).T)

    in_maps = []
    for k in range(NCORES):
        in_maps.append({
            "x": np.ascontiguousarray(x[k * NL:(k + 1) * NL]),
            "wct": wct, "emat": emat, "ematt": ematt, "fc2t": fc2t,
        })
    return in_maps


def _run(inputs: dict, trace: bool = False):
    in_maps = _prep_inputs(**inputs)
    nc = _get_nc()
    res = run_bass_kernel_spmd(nc, in_maps, list(range(NCORES)), trace=trace)
    out = np.concatenate([res.results[k]["out"] for k in range(NCORES)], axis=0)
    return out.reshape(N, C, 1, 1).astype(np.float32), res


def kernel(**inputs) -> np.ndarray:
    out, _ = _run(inputs, trace=False)
    return out


# revision 12
# speedup vs baseline: 1.1229x; 1.1229x over previous
"""Trainium2 Bass kernel for nn_CGCA_branch (gnn_message_passing).

Math: the reference applies 1x1 convs (C->CA, grouped CA->CA), global average
pool, fc1, adjacency-softmax matmul, relu, fc2, sigmoid.  Every op between x
and the relu is linear, and the global average pool commutes with the 1x1
convs, so the whole prefix collapses to

    f1[n, :] = Wcomb @ sum_s(x[n, :, s]),   Wcomb = fc1_w @ M2 @ (w1 / S)

with M2 the block-diagonal form of the grouped conv.  The kernel therefore
only needs a 411 MB spatial-sum reduction of x (HBM-bound) plus tiny matmuls.

Sharding: pure data parallel - batch 64 split into 8 shards of 8 samples,
one per NeuronCore; weights replicated.
"""

import numpy as np

import concourse.bass as bass
import concourse.bacc as bacc
import concourse.tile as tile
from concourse import mybir
from concourse.bass_utils import run_bass_kernel_spmd
from concourse.tile import TileContext
from contextlib import ExitStack

# ---- problem constants (hardcoded per harness contract) ----
N, C, H, W = 64, 512, 56, 56
S = H * W                      # 3136 spatial positions
J, CA, G = 17, 272, 16
NCORES = 8
NL = N // NCORES               # 8 samples per core
CT = C // 128                  # 4 channel chunks of 128
NEG = -9e15

_ADJ = np.array([
    [1,1,0,0,0,0,0,0,0,0,0,0,0,0,0,0,0],[1,1,1,0,0,0,0,0,0,0,0,0,0,0,0,0,0],
    [0,1,1,0,0,0,1,0,0,0,0,0,0,0,0,0,0],[0,0,0,1,1,0,1,0,0,0,0,0,0,0,0,0,0],
    [0,0,0,1,1,1,0,0,0,0,0,0,0,0,0,0,0],[0,0,0,0,1,1,0,0,0,0,0,0,0,0,0,0,0],
    [0,0,1,1,0,0,1,1,0,0,0,0,0,0,0,0,0],[0,0,0,0,0,0,1,1,1,0,0,0,0,0,0,0,0],
    [0,0,0,0,0,0,0,1,1,0,0,1,1,0,0,0,1],[0,0,0,0,0,0,0,0,0,1,0,0,0,0,0,0,1],
    [0,0,0,0,0,0,0,0,0,0,1,1,0,0,0,0,0],[0,0,0,0,0,0,0,0,0,0,1,1,1,0,0,0,0],
    [0,0,0,0,0,0,0,0,1,0,0,1,1,0,0,0,0],[0,0,0,0,0,0,0,0,1,0,0,0,0,1,1,0,0],
    [0,0,0,0,0,0,0,0,0,0,0,0,0,1,1,1,0],[0,0,0,0,0,0,0,0,0,0,0,0,0,0,1,1,0],
    [0,0,0,0,0,0,0,0,1,1,0,0,0,0,0,0,1]], dtype=np.int32)
NZ_IDX = np.flatnonzero(_ADJ)  # 49 entries

F32 = mybir.dt.float32
_NC_CACHE = {}


def _build_nc() -> bass.Bass:
    nc = bacc.Bacc(None, enable_partition_id=False)
    x_d = nc.declare_dram_parameter("x", [NL, C, S], F32, isOutput=False)
    wct_d = nc.declare_dram_parameter("wct", [128, CT, J], F32, isOutput=False)
    emat_d = nc.declare_dram_parameter("emat", [J, J], F32, isOutput=False)
    ematt_d = nc.declare_dram_parameter("ematt", [J, J], F32, isOutput=False)
    fc2t_d = nc.declare_dram_parameter("fc2t", [J, C], F32, isOutput=False)
    out_d = nc.declare_dram_parameter("out", [NL, C], F32, isOutput=True)

    with TileContext(nc) as tc, ExitStack() as ctx:
        xpool = ctx.enter_context(tc.tile_pool(name="xpool", bufs=6))
        singles = ctx.enter_context(tc.tile_pool(name="singles", bufs=1))
        smalls = ctx.enter_context(tc.tile_pool(name="smalls", bufs=3))
        resp = ctx.enter_context(tc.tile_pool(name="resp", bufs=NL))
        psum = ctx.enter_context(tc.tile_pool(name="psum", bufs=2, space="PSUM"))

        # ---- replicated weights / adjacency prep (tiny, one-time).
        # SWDGE queue so the SP HWDGE ring carries only the x stream.
        wct_sb = singles.tile([128, CT, J], F32)
        nc.gpsimd.dma_start(out=wct_sb, in_=wct_d[:, :, :])
        fc2t_sb = singles.tile([J, C], F32)
        nc.gpsimd.dma_start(out=fc2t_sb, in_=fc2t_d[:, :])
        e_sb = singles.tile([J, J], F32)
        nc.gpsimd.dma_start(out=e_sb, in_=emat_d[:, :])
        et_sb = singles.tile([J, J], F32)
        nc.gpsimd.dma_start(out=et_sb, in_=ematt_d[:, :])

        # softmax over rows of E: adj[i,j] = exp(E[i,j]) / rs[i].
        # We keep exp(E^T) as the matmul lhsT and fold 1/rs in afterwards.
        a_sb = singles.tile([J, J], F32)
        nc.scalar.activation(out=a_sb, in_=e_sb,
                             func=mybir.ActivationFunctionType.Exp)
        at_sb = singles.tile([J, J], F32)
        nc.scalar.activation(out=at_sb, in_=et_sb,
                             func=mybir.ActivationFunctionType.Exp)
        rs_sb = singles.tile([J, 1], F32)
        nc.vector.reduce_sum(out=rs_sb, in_=a_sb, axis=mybir.AxisListType.X)
        rrs_sb = singles.tile([J, 1], F32)
        nc.vector.reciprocal(out=rrs_sb, in_=rs_sb)

        # ---- stream x, spatial-sum per (sample, channel-chunk) ----
        # The tail-critical final chunks are split into smaller pieces so the
        # last reduce after the last DMA is ~1us instead of ~3us; the partial
        # sums are folded into extra PSUM-accumulated matmuls below.
        xm_sb = singles.tile([128, CT, NL], F32)        # xm[p, ct, n]
        stage = singles.tile([128, 8], F32)             # split-piece partials
        scratch = singles.tile([128, S], F32)           # dummy out for ACT accum
        xv = x_d[:, :, :].rearrange("n (ct p) s -> n p ct s", p=128)

        n_pieces = {(NL - 1, CT - 2): 2, (NL - 1, CT - 1): 4}
        f1_ops = {n: [] for n in range(NL)}   # (lhsT, rhs) accumulation list
        stage_col = 0

        def emit_reduce(n, ct, use_dve):
            nonlocal stage_col
            pieces = n_pieces.get((n, ct), 1)
            w = S // pieces
            for pi in range(pieces):
                xt = xpool.tile([128, w], F32, tag="xt")
                nc.sync.dma_start(out=xt, in_=xv[n, :, ct, pi * w:(pi + 1) * w])
                if pieces == 1:
                    dst = xm_sb[:, ct, n:n + 1]
                else:
                    dst = stage[:, stage_col:stage_col + 1]
                    stage_col += 1
                f1_ops[n].append((wct_sb[:, ct, :], dst))
                if use_dve:
                    nc.vector.reduce_sum(out=dst, in_=xt,
                                         axis=mybir.AxisListType.X)
                else:
                    nc.scalar.activation(
                        out=scratch[:, :w], in_=xt,
                        func=mybir.ActivationFunctionType.Copy,
                        accum_out=dst)

        def emit_sample_chain(n):
            # f1[:, n] = Wcomb @ xm[:, n]  (accumulate over chunks/pieces)
            ops = f1_ops[n]
            f1_ps = psum.tile([J, 1], F32, tag="f1")
            for i, (lhsT, rhs) in enumerate(ops):
                nc.tensor.matmul(f1_ps, lhsT=lhsT, rhs=rhs,
                                 start=(i == 0), stop=(i == len(ops) - 1))
            f1_sb = smalls.tile([J, 1], F32, tag="f1s")
            nc.scalar.copy(out=f1_sb, in_=f1_ps)
            # gc = relu(adj @ f1) via exp(E^T) lhsT and 1/rs scaling
            gc_ps = psum.tile([J, 1], F32, tag="gc")
            nc.tensor.matmul(gc_ps, lhsT=at_sb, rhs=f1_sb, start=True,
                             stop=True)
            gc_sb = smalls.tile([J, 1], F32, tag="gcs")
            nc.vector.tensor_scalar(out=gc_sb, in0=gc_ps, scalar1=rrs_sb,
                                    scalar2=0.0, op0=mybir.AluOpType.mult,
                                    op1=mybir.AluOpType.max)
            # out = sigmoid(gc.T @ fc2t), in two column halves so the final
            # matmul / sigmoid / store pipeline on the kernel tail
            res_sb = resp.tile([1, C], F32, tag="res")
            half = C // 2
            for h in range(2):
                o_ps = psum.tile([1, half], F32, tag="o")
                nc.tensor.matmul(o_ps, lhsT=gc_sb,
                                 rhs=fc2t_sb[:, h * half:(h + 1) * half],
                                 start=True, stop=True)
                nc.scalar.activation(out=res_sb[:, h * half:(h + 1) * half],
                                     in_=o_ps,
                                     func=mybir.ActivationFunctionType.Sigmoid)
            return res_sb

        results = []
        for n in range(NL):
            for ct in range(CT):
                emit_reduce(n, ct, use_dve=(ct % 2 == 0))
            results.append(emit_sample_chain(n))
        # output DMAs last so they never stall the x stream on the SP ring
        for n, res_sb in enumerate(results):
            nc.sync.dma_start(out=out_d[n:n + 1, :], in_=res_sb)

    return nc


def _get_nc() -> bass.Bass:
    if "nc" not in _NC_CACHE:
        nc = _build_nc()
        nc.finalize()
        _NC_CACHE["nc"] = nc
    return _NC_CACHE["nc"]


def _prep_inputs(x, e, w1, w2, fc1_w, fc2_w):
    """Host-side shard + weight fold (layout prep only; heavy math on device)."""
    x = np.ascontiguousarray(np.asarray(x, dtype=np.float32)).reshape(N, C, S)

    # fold conv1 / grouped-conv2 / fc1 / (1/S mean) into one [J, C] matrix
    w1d = np.asarray(w1, dtype=np.float64)
    w2g = np.asarray(w2, dtype=np.float64).reshape(G, J, J)
    m2 = np.zeros((CA, CA), dtype=np.float64)
    for g in range(G):
        m2[g * J:(g + 1) * J, g * J:(g + 1) * J] = w2g[g]
    wcomb = np.asarray(fc1_w, np.float64) @ m2 @ (w1d / S)      # [J, C]
    wct = np.ascontiguousarray(
        wcomb.T.reshape(CT, 128, J).transpose(1, 0, 2)).astype(np.float32)

    emat = np.full((J * J,), NEG, dtype=np.float32)
    emat[NZ_IDX] = np.asarray(e, dtype=np.float32)[0]
    emat = emat.reshape(J, J)
    ematt = np.ascontiguousarray(emat.T)
    fc2t = np.ascontiguousarray(np.asarray(fc2_w, dtype=np.float32).T)

    in_maps = []
    for k in range(NCORES):
        in_maps.append({
            "x": np.ascontiguousarray(x[k * NL:(k + 1) * NL]),
            "wct": wct, "emat": emat, "ematt": ematt, "fc2t": fc2t,
        })
    return in_maps


def _run(inputs: dict, trace: bool = False):
    in_maps = _prep_inputs(**inputs)
    nc = _get_nc()
    res = run_bass_kernel_spmd(nc, in_maps, list(range(NCORES)), trace=trace)
    out = np.concatenate([res.results[k]["out"] for k in range(NCORES)], axis=0)
    return out.reshape(N, C, 1, 1).astype(np.float32), res


def kernel(**inputs) -> np.ndarray:
    out, _ = _run(inputs, trace=False)
    return out
